# revision 51
# baseline (speedup 1.0000x reference)
"""Trainium2 Bass kernel for FPModule (knn_interpolate + MLP).

Takes FULL unsharded inputs, shards data-parallel over the M=16384 query
points across 8 NeuronCores, returns the FULL [16384, 256] output.

Per-core device algorithm (M_loc = 2048 queries, N = 4096 coarse points,
16 m-tiles of 128 queries):
  1. PE: S[m, n] = -d2[m, n] via a contract-45 bf16 matmul: every fp32
     operand (2q_c, -q_c^2, p_c, -p_c^2 per coordinate) is decomposed
     hi/mid/lo into three bf16 values; all cross terms are accumulated in
     an order that cancels within each coordinate, so S carries fp32-grade
     accuracy at bf16 matmul speed (bf16 products are exact in fp32, and
     the PSUM partials stay near d2's own scale).
  2. ACT: drain PSUM -> SBUF.
  3. DVE: max/max_index (hardware top-8 per partition, one pass each) ->
     top-3 neighbor values + indices; d2 = -vals, clamp 1e-16,
     inverse-distance weights normalized on-chip (this is the bottleneck
     engine: 2 x 4.42us full-row scans per tile is its hard floor).
  4. GPSIMD SWDGE: indirect-DMA row gathers of x[idx] from HBM (pools deep
     enough that the gathers' only sync-wait is the genuine idx8
     dependency, keeping the queue drain-free); ACT applies the
     per-partition weights (xgw_k = xg_k * wn_k).
  5. PE: yT = sum_k transpose(xgw_k) via pure accumulating transposes
     (bit-exact), then the 2-layer MLP per tile with the contract dim on
     partitions; b1 rides the ReLU bias, b2 is a rank-2 bf16 matmul. All
     per-tile work is emitted inline so it pipelines under the DVE scans.

Constants come from host-packed blobs ordered hot-first so PE starts
early; catT/h1T are per-tile tiles because Tile tracks dependencies at
tile granularity; a post-pass splits residual multi-sync-waits onto
same-engine drains (this walrus encodes only one sync-wait per TPB
instruction).
"""

from contextlib import ExitStack

import numpy as np

import concourse.bass as bass
import concourse.mybir as mybir
import concourse.tile as tile
from concourse.bass import IndirectOffsetOnAxis

F32 = mybir.dt.float32
BF16 = mybir.dt.bfloat16
U32 = mybir.dt.uint32

N_CORES = 8
N = 4096          # coarse points
C_IN = 256        # x feature dim
C_SKIP = 128      # x_skip feature dim
HID = 256         # MLP hidden/output dim
M = 16384         # query points
ML = M // N_CORES # queries per core
P = 128           # partitions
T = ML // P       # m-tiles per core (16)
EPS = 1e-16
CR = 45           # rank-matmul contract dim: 15 terms per coordinate

# hot fp32 blob: [128, 146]
OFF_QQ = 0                  # [128, 16] |q|^2 per m-tile column
OFF_ID = OFF_QQ + T         # [128, 128] identity
OFF_B1C = OFF_ID + P        # [128, 2]  b1 per-partition column per hc
BLOBH_F = OFF_B1C + 2

# cold fp32 blob: [128, 3328]
OFF_W1 = 0                  # 3 x [128, 256]
OFF_W2 = OFF_W1 + 3 * HID   # 2 x [128, 256]
OFF_XST = OFF_W2 + 2 * HID  # [128, 2048] x_skip^T
BLOBW_F = OFF_XST + ML

# bf16 rank blobs. Split in two so the first (queries + first coarse
# chunks) lands fast and PE starts within ~2us of kernel entry.
OFF_AQ = 0                  # [30, 2048] split query operand
OFF_AP0 = OFF_AQ + ML       # [30, 1024] coarse chunks 0-1
BLOB16A_F = OFF_AP0 + 1024
BLOB16C_F = N - 1024        # [30, 3072] coarse chunks 2-7

# bf16 bias blob: [2, 384]
OFF_ON2 = 0                 # [2, 128] ones (b2 lhsT)
OFF_B2 = OFF_ON2 + P        # [2, 256] b2 hi/lo
BLOB16B_F = OFF_B2 + HID

# bf16 starter blob: tile-0 query operand + coarse chunk 0, so the very
# first rank matmuls launch after a ~0.5us DMA instead of the full blob
BLOB16Z_F = P + 512

# Instruction types that never reach walrus's TPB sync-wait encoding.
_SAFE_MULTIWAIT = {
    "InstUnconditionalBranch", "InstCall", "InstRegisterMove",
}


def _legalize_waits(nc: bass.Bass, budget: int = 1) -> None:
    """Walrus TPB instruction encodings only fit `budget` sync-waits; move
    any excess onto same-engine Drains inserted just before (semantically
    identical: waits are stalls, and same-engine program order is kept)."""
    for blk in nc.m.functions[0].blocks:
        out = []
        for ins in blk.instructions:
            si = ins.sync_info
            if (si is not None and len(si.on_wait) > budget
                    and type(ins).__name__ not in _SAFE_MULTIWAIT):
                extra = list(si.on_wait[:-budget])
                keep = list(si.on_wait[-budget:])
                for w in extra:
                    out.append(mybir.InstDrain(
                        name=f"I-lw{nc.next_id()}",
                        engine=ins.engine,
                        debug=ins.debug,
                        sync_info=mybir.SyncInfo(on_wait=[w], on_update=[]),
                    ))
                si.on_wait = keep
            out.append(ins)
        blk.instructions[:] = out


def build_program(legalize: bool = True) -> bass.Bass:
    nc = bass.Bass("TRN2", target_bir_lowering=False, debug=False,
                   num_devices=N_CORES, num_swdge_queues=4)

    x_d = nc.dram_tensor("x", [N, C_IN], F32, kind="ExternalInput")
    blobh_d = nc.dram_tensor("blobh", [P, BLOBH_F], F32, kind="ExternalInput")
    blobw_d = nc.dram_tensor("blobw", [P, BLOBW_F], F32, kind="ExternalInput")
    blob16a_d = nc.dram_tensor("blob16a", [CR, BLOB16A_F], BF16,
                               kind="ExternalInput")
    blob16c_d = nc.dram_tensor("blob16c", [CR, BLOB16C_F], BF16,
                               kind="ExternalInput")
    blob16b_d = nc.dram_tensor("blob16b", [2, BLOB16B_F], BF16,
                               kind="ExternalInput")
    blob16z_d = nc.dram_tensor("blob16z", [CR, BLOB16Z_F], BF16,
                               kind="ExternalInput")
    out_d = nc.dram_tensor("out", [ML, HID], F32, kind="ExternalOutput")

    with ExitStack() as ctx:
        tc = ctx.enter_context(tile.TileContext(nc))
        consts = ctx.enter_context(tc.tile_pool(name="consts", bufs=1))
        srow_p = ctx.enter_context(tc.tile_pool(name="srow", bufs=3))
        ps_d2 = ctx.enter_context(tc.tile_pool(name="ps_d2", bufs=2, space="PSUM"))
        ps_sm = ctx.enter_context(tc.tile_pool(name="ps_sm", bufs=4, space="PSUM"))
        small = ctx.enter_context(tc.tile_pool(name="small", bufs=16))
        gath = ctx.enter_context(tc.tile_pool(name="gath", bufs=4))
        outp = ctx.enter_context(tc.tile_pool(name="outp", bufs=4))

        # load order matters: the tiny bf16 rank blobs come first so PE can
        # start immediately; the big weight blob is only needed later.
        blob16z = consts.tile([CR, BLOB16Z_F], BF16, tag="blob16z")
        nc.sync.dma_start(blob16z[:], blob16z_d[:])
        blob16a = consts.tile([CR, BLOB16A_F], BF16, tag="blob16a")
        nc.sync.dma_start(blob16a[:], blob16a_d[:])
        blob16c = consts.tile([CR, BLOB16C_F], BF16, tag="blob16c")
        nc.sync.dma_start(blob16c[:], blob16c_d[:])
        blobh = consts.tile([P, BLOBH_F], F32, tag="blobh")
        nc.sync.dma_start(blobh[:], blobh_d[:])
        blob16b = consts.tile([2, BLOB16B_F], BF16, tag="blob16b")
        nc.sync.dma_start(blob16b[:], blob16b_d[:])
        blobw = consts.tile([P, BLOBW_F], F32, tag="blobw")
        nc.sync.dma_start(blobw[:], blobw_d[:])

        ident = blobh[0:P, OFF_ID:OFF_ID + P]
        b1c = lambda hc: blobh[0:P, OFF_B1C + hc:OFF_B1C + hc + 1]
        w1 = lambda c, hc: blobw[0:P, OFF_W1 + c * HID + hc * P:
                                 OFF_W1 + c * HID + (hc + 1) * P]
        w2 = lambda hc: blobw[0:P, OFF_W2 + hc * HID:OFF_W2 + (hc + 1) * HID]
        xst = lambda i: blobw[0:P, OFF_XST + i * P:OFF_XST + (i + 1) * P]
        def augp(j, i=1):
            if i == 0 and j == 0:
                return blob16z[0:CR, P:P + 512]
            if j < 2:
                return blob16a[0:CR, OFF_AP0 + j * 512:OFF_AP0 + (j + 1) * 512]
            return blob16c[0:CR, (j - 2) * 512:(j - 1) * 512]

        augq = lambda i: (blob16z[0:CR, 0:P] if i == 0 else
                          blob16a[0:CR, OFF_AQ + i * P:OFF_AQ + (i + 1) * P])
        ones2 = blob16b[0:2, OFF_ON2:OFF_ON2 + P]
        b2hl = blob16b[0:2, OFF_B2:OFF_B2 + HID]

        # per-tile tiles: Tile tracks dependencies at tile granularity, so
        # chunk-wide tensors would stall each tile's MLP on its neighbors
        catT = [[consts.tile([P, P], F32, tag=f"catT_{c}_{i}",
                             name=f"catT_{c}_{i}")
                 for i in range(T)] for c in range(2)]
        h1T = [[consts.tile([P, P], F32, tag=f"h1T_{hc}_{i}",
                            name=f"h1T_{hc}_{i}")
                for i in range(T)] for hc in range(2)]

        # ---- main loop over m-tiles, MLP interleaved per 4-tile chunk ----
        for i in range(T):
            # S = 2 q.p - |p|^2 for this tile's 128 queries x all 4096 points
            srow = srow_p.tile([P, N], F32, tag="srow")
            for j in range(N // 1024):
                ps = ps_d2.tile([P, 1024], F32, tag="ps")
                for h in range(2):
                    nc.tensor.matmul(ps[:, h * 512:(h + 1) * 512],
                                     lhsT=augq(i), rhs=augp(2 * j + h, i),
                                     start=True, stop=True)
                if i == 0:
                    # fine-grained drains on the first tile shorten the
                    # serial startup chain to the first MAX8
                    for h in range(2):
                        sl = slice((2 * j + h) * 512, (2 * j + h + 1) * 512)
                        nc.scalar.copy(srow[:, sl], ps[:, h * 512:(h + 1) * 512])
                else:
                    nc.scalar.copy(srow[:, j * 1024:(j + 1) * 1024], ps[:])

            # top-8 (we use top-3) by S descending == nearest by d2
            vals8 = small.tile([P, 8], F32, tag="vals8")
            idx8 = small.tile([P, 8], U32, tag="idx8")
            nc.vector.max(out=vals8[:], in_=srow[:])
            nc.vector.max_index(out=idx8[:], in_max=vals8[:], in_values=srow[:])

            # the matmul computes -d2 directly, so d2 = -vals
            d2s = small.tile([P, 3], F32, tag="d2s")
            nc.scalar.activation(d2s[:], vals8[:, 0:3],
                                 mybir.ActivationFunctionType.Identity,
                                 scale=-1.0)
            nc.vector.tensor_scalar_max(d2s[:], d2s[:], EPS)
            w3 = small.tile([P, 3], F32, tag="w3")
            nc.vector.reciprocal(w3[:], d2s[:])
            wsum = small.tile([P, 1], F32, tag="wsum")
            nc.vector.reduce_sum(wsum[:], w3[:], axis=mybir.AxisListType.X)
            winv = small.tile([P, 1], F32, tag="winv")
            nc.vector.reciprocal(winv[:], wsum[:])
            wn = small.tile([P, 3], F32, tag="wn")
            nc.scalar.activation(wn[:], w3[:],
                                 mybir.ActivationFunctionType.Copy,
                                 scale=winv[:])

            # gather the 3 neighbor rows; xg pools are deep enough that slots
            # are never reused, so each gather's only wait is the genuine DVE
            # idx8 dependency (keeps the SWDGE queue drain-free). Weighting
            # on ACT (per-partition scale).
            xgws = []
            for k in range(3):
                xg = gath.tile([P, C_IN], F32, tag=f"xg{k}", bufs=16,
                               name=f"xg{k}")
                gi = nc.gpsimd.indirect_dma_start(
                    out=xg[:], out_offset=None, in_=x_d[:],
                    in_offset=IndirectOffsetOnAxis(ap=idx8[:, k:k + 1], axis=0))
                # spread the gathers over the 4 SWDGE queues so the three
                # per-tile gathers run concurrently
                gi.ins.queue = f"qPoolDynamic{k or ''}"
                xgw = gath.tile([P, C_IN], F32, tag=f"xgw{k}", name=f"xgw{k}")
                nc.scalar.activation(xgw[:], xg[:],
                                     mybir.ActivationFunctionType.Copy,
                                     scale=wn[:, k:k + 1])
                xgws.append(xgw)

            # yT = sum_k transpose(xgw_k): pure accumulating PE transposes
            for h in range(2):
                pt = ps_sm.tile([P, P], F32, tag="sm")
                for k in range(3):
                    nc.tensor.matmul(pt[:], lhsT=xgws[k][:, h * P:(h + 1) * P],
                                     rhs=ident, is_transpose=True,
                                     start=(k == 0), stop=(k == 2))
                nc.scalar.copy(catT[h][i][:], pt[:])

            # MLP for this tile's 128 queries, emitted inline so PE/ACT load
            # stays even across tiles and nothing piles up at the end.
            for hc in range(2):
                ph = ps_sm.tile([P, 512], F32, tag="sm")
                for c in range(3):
                    rhs = catT[c][i][:] if c < 2 else xst(i)
                    nc.tensor.matmul(ph[:, :P], lhsT=w1(c, hc), rhs=rhs,
                                     start=(c == 0), stop=(c == 2))
                nc.scalar.activation(
                    h1T[hc][i][:], ph[:, :P],
                    mybir.ActivationFunctionType.Relu, bias=b1c(hc))
            po = ps_sm.tile([P, HID], F32, tag="sm")
            for hc in range(2):
                nc.tensor.matmul(
                    po[:], lhsT=h1T[hc][i][:],
                    rhs=w2(hc), start=(hc == 0), stop=False)
            nc.tensor.matmul(po[:], lhsT=ones2, rhs=b2hl,
                             start=False, stop=True)
            ob = outp.tile([P, HID], F32, tag="ob")
            nc.scalar.copy(ob[:], po[:])
            nc.sync.dma_start(out_d[i * P:(i + 1) * P, :], ob[:])

    if legalize:
        _legalize_waits(nc)
    return nc


def _split3(a):
    """fp32 -> (hi, mid, lo) bf16 triplet with hi+mid+lo ~= a to ~2^-25."""
    import ml_dtypes
    bf = ml_dtypes.bfloat16
    h = a.astype(bf)
    r = a - h.astype(np.float32)
    m = r.astype(bf)
    l = (r - m.astype(np.float32)).astype(bf)
    return h, m, l


def make_in_maps(x, pos, x_skip, pos_skip):
    """Host-side prep: split operands + per-core packed blobs."""
    import ml_dtypes
    bf = ml_dtypes.bfloat16

    x = np.ascontiguousarray(np.asarray(x, np.float32))
    pos = np.asarray(pos, np.float32)
    x_skip = np.asarray(x_skip, np.float32)
    pos_skip = np.asarray(pos_skip, np.float32)

    # The rank matmul computes S = -d2 = sum_c -(q_c - p_c)^2 with a 15-term
    # expansion per coordinate: hi/mid/lo bf16 splits of 2q_c, -q_c^2, p_c,
    # -p_c^2, ordered so the running partial cancels within each magnitude
    # level (keeps the fp32 PSUM accumulation noise near 1 ulp of d2's
    # scale instead of 1 ulp of |2q.p|).
    pc = [pos[:, c] for c in range(3)]
    psp = [_split3(p) for p in pc]                    # p_c splits
    npp = [_split3(-(p * p)) for p in pc]             # -p_c^2 splits
    onep = np.ones(N, np.float32).astype(bf)

    def coord_rows(qs2, nqq, ps, nps, onq, onp):
        """15 (q_row, p_row) pairs for one coordinate, small-partial order.
        qs2 = splits of 2q_c, nqq = splits of -q_c^2, ps = splits of p_c,
        nps = splits of -p_c^2, onq/onp = ones rows."""
        return [
            (qs2[0], ps[0]), (onq, nps[0]), (nqq[0], onp),      # hi level
            (qs2[0], ps[1]), (qs2[1], ps[0]), (onq, nps[1]),    # mid cross
            (nqq[1], onp), (qs2[1], ps[1]),
            (qs2[0], ps[2]), (qs2[2], ps[0]), (onq, nps[2]),    # lo cross
            (nqq[2], onp), (qs2[1], ps[2]), (qs2[2], ps[1]),
            (qs2[2], ps[2]),
        ]

    in_maps = []
    for core in range(N_CORES):
        sl = slice(core * ML, (core + 1) * ML)
        q = pos_skip[sl]
        qc = [q[:, c] for c in range(3)]
        qsp2 = [_split3(2.0 * qv) for qv in qc]
        nqq = [_split3(-(qv * qv)) for qv in qc]
        oneq = np.ones(ML, np.float32).astype(bf)

        aq_rows, ap_rows = [], []
        for c in range(3):
            for qr, pr in coord_rows(qsp2[c], nqq[c], psp[c], npp[c],
                                     oneq, onep):
                aq_rows.append(qr)
                ap_rows.append(pr)

        blobh = np.zeros((P, BLOBH_F), np.float32)
        blobh[0:P, OFF_ID:OFF_ID + P] = np.eye(P, dtype=np.float32)

        blobw = np.zeros((P, BLOBW_F), np.float32)
        blobw[0:P, OFF_XST:OFF_XST + ML] = x_skip[sl].T

        ap_full = np.stack(ap_rows)
        blob16a = np.zeros((CR, BLOB16A_F), bf)
        blob16a[0:CR, OFF_AQ:OFF_AQ + ML] = np.stack(aq_rows)
        blob16a[0:CR, OFF_AP0:OFF_AP0 + 1024] = ap_full[:, 0:1024]
        blob16c = np.ascontiguousarray(ap_full[:, 1024:N])

        blob16b = np.zeros((2, BLOB16B_F), bf)
        blob16b[0:2, OFF_ON2:OFF_ON2 + P] = np.ones((2, P), bf)

        blob16z = np.zeros((CR, BLOB16Z_F), bf)
        blob16z[0:CR, 0:P] = blob16a[0:CR, OFF_AQ:OFF_AQ + P]
        blob16z[0:CR, P:P + 512] = ap_full[:, 0:512]

        in_maps.append({"x": x, "blobh": blobh, "blobw": blobw,
                        "blob16a": blob16a, "blob16c": blob16c,
                        "blob16b": blob16b, "blob16z": blob16z})
    return in_maps


def fill_weights(in_maps, W1, b1, W2, b2):
    import ml_dtypes
    bf = ml_dtypes.bfloat16
    W1 = np.asarray(W1, np.float32)
    W2 = np.asarray(W2, np.float32)
    b1 = np.asarray(b1, np.float32).reshape(-1)
    b2 = np.asarray(b2, np.float32).reshape(-1)
    b2h = b2.astype(bf)
    b2l = (b2 - b2h.astype(np.float32)).astype(bf)
    for m in in_maps:
        blobw = m["blobw"]
        for c in range(3):
            blobw[0:P, OFF_W1 + c * HID:OFF_W1 + (c + 1) * HID] = \
                W1[c * P:(c + 1) * P, :]
        for hc in range(2):
            blobw[0:P, OFF_W2 + hc * HID:OFF_W2 + (hc + 1) * HID] = \
                W2[hc * P:(hc + 1) * P, :]
            m["blobh"][0:P, OFF_B1C + hc] = b1[hc * P:(hc + 1) * P]
        m["blob16b"][0:1, OFF_B2:OFF_B2 + HID] = b2h
        m["blob16b"][1:2, OFF_B2:OFF_B2 + HID] = b2l
    return in_maps


_NC_CACHE = {}


def kernel(x, pos, x_skip, pos_skip, W1, b1, W2, b2):
    from concourse.bass_utils import run_bass_kernel_spmd

    if "nc" not in _NC_CACHE:
        _NC_CACHE["nc"] = build_program()
    nc = _NC_CACHE["nc"]

    in_maps = make_in_maps(x, pos, x_skip, pos_skip)
    fill_weights(in_maps, W1, b1, W2, b2)

    res = run_bass_kernel_spmd(nc, in_maps, list(range(N_CORES))).results
    out = np.concatenate([res[c]["out"] for c in range(N_CORES)], axis=0)
    return out.astype(np.float32)


# revision 52
# speedup vs baseline: 1.0038x; 1.0038x over previous
"""Trainium2 Bass kernel for FPModule (knn_interpolate + MLP).

Takes FULL unsharded inputs, shards data-parallel over the M=16384 query
points across 8 NeuronCores, returns the FULL [16384, 256] output.

Per-core device algorithm (M_loc = 2048 queries, N = 4096 coarse points,
16 m-tiles of 128 queries):
  1. PE: S[m, n] = -d2[m, n] via a contract-45 bf16 matmul: every fp32
     operand (2q_c, -q_c^2, p_c, -p_c^2 per coordinate) is decomposed
     hi/mid/lo into three bf16 values; all cross terms are accumulated in
     an order that cancels within each coordinate, so S carries fp32-grade
     accuracy at bf16 matmul speed (bf16 products are exact in fp32, and
     the PSUM partials stay near d2's own scale).
  2. ACT: drain PSUM -> SBUF.
  3. DVE: max/max_index (hardware top-8 per partition, one pass each) ->
     top-3 neighbor values + indices; d2 = -vals, clamp 1e-16,
     inverse-distance weights normalized on-chip (this is the bottleneck
     engine: 2 x 4.42us full-row scans per tile is its hard floor).
  4. GPSIMD SWDGE: indirect-DMA row gathers of x[idx] from HBM (pools deep
     enough that the gathers' only sync-wait is the genuine idx8
     dependency, keeping the queue drain-free); ACT applies the
     per-partition weights (xgw_k = xg_k * wn_k).
  5. PE: yT = sum_k transpose(xgw_k) via pure accumulating transposes
     (bit-exact), then the 2-layer MLP per tile with the contract dim on
     partitions; b1 rides the ReLU bias, b2 is a rank-2 bf16 matmul. All
     per-tile work is emitted inline so it pipelines under the DVE scans.

Constants come from host-packed blobs ordered hot-first so PE starts
early; catT/h1T are per-tile tiles because Tile tracks dependencies at
tile granularity; a post-pass splits residual multi-sync-waits onto
same-engine drains (this walrus encodes only one sync-wait per TPB
instruction).
"""

from contextlib import ExitStack

import numpy as np

import concourse.bass as bass
import concourse.mybir as mybir
import concourse.tile as tile
from concourse.bass import IndirectOffsetOnAxis

F32 = mybir.dt.float32
BF16 = mybir.dt.bfloat16
U32 = mybir.dt.uint32

N_CORES = 8
N = 4096          # coarse points
C_IN = 256        # x feature dim
C_SKIP = 128      # x_skip feature dim
HID = 256         # MLP hidden/output dim
M = 16384         # query points
ML = M // N_CORES # queries per core
P = 128           # partitions
T = ML // P       # m-tiles per core (16)
EPS = 1e-16
CR = 45           # rank-matmul contract dim: 15 terms per coordinate

# hot fp32 blob: [128, 146]
OFF_QQ = 0                  # [128, 16] |q|^2 per m-tile column
OFF_ID = OFF_QQ + T         # [128, 128] identity
OFF_B1C = OFF_ID + P        # [128, 2]  b1 per-partition column per hc
BLOBH_F = OFF_B1C + 2

# cold fp32 blob: [128, 3328]
OFF_W1 = 0                  # 3 x [128, 256]
OFF_W2 = OFF_W1 + 3 * HID   # 2 x [128, 256]
OFF_XST = OFF_W2 + 2 * HID  # [128, 2048] x_skip^T
BLOBW_F = OFF_XST + ML

# bf16 rank blobs. Split in two so the first (queries + first coarse
# chunks) lands fast and PE starts within ~2us of kernel entry.
OFF_AQ = 0                  # [30, 2048] split query operand
OFF_AP0 = OFF_AQ + ML       # [30, 1024] coarse chunks 0-1
BLOB16A_F = OFF_AP0 + 1024
BLOB16C_F = N - 1024        # [30, 3072] coarse chunks 2-7

# bf16 bias blob: [2, 384]
OFF_ON2 = 0                 # [2, 128] ones (b2 lhsT)
OFF_B2 = OFF_ON2 + P        # [2, 256] b2 hi/lo
BLOB16B_F = OFF_B2 + HID

# bf16 starter blob: tile-0 query operand + coarse chunk 0, so the very
# first rank matmuls launch after a ~0.5us DMA instead of the full blob
BLOB16Z_F = P + 512

# Instruction types that never reach walrus's TPB sync-wait encoding.
_SAFE_MULTIWAIT = {
    "InstUnconditionalBranch", "InstCall", "InstRegisterMove",
}


def _legalize_waits(nc: bass.Bass, budget: int = 1) -> None:
    """Walrus TPB instruction encodings only fit `budget` sync-waits; move
    any excess onto same-engine Drains inserted just before (semantically
    identical: waits are stalls, and same-engine program order is kept)."""
    for blk in nc.m.functions[0].blocks:
        out = []
        for ins in blk.instructions:
            si = ins.sync_info
            if (si is not None and len(si.on_wait) > budget
                    and type(ins).__name__ not in _SAFE_MULTIWAIT):
                extra = list(si.on_wait[:-budget])
                keep = list(si.on_wait[-budget:])
                for w in extra:
                    out.append(mybir.InstDrain(
                        name=f"I-lw{nc.next_id()}",
                        engine=ins.engine,
                        debug=ins.debug,
                        sync_info=mybir.SyncInfo(on_wait=[w], on_update=[]),
                    ))
                si.on_wait = keep
            out.append(ins)
        blk.instructions[:] = out


def build_program(legalize: bool = True) -> bass.Bass:
    nc = bass.Bass("TRN2", target_bir_lowering=False, debug=False,
                   num_devices=N_CORES, num_swdge_queues=4)

    x_d = nc.dram_tensor("x", [N, C_IN], F32, kind="ExternalInput")
    blobh_d = nc.dram_tensor("blobh", [P, BLOBH_F], F32, kind="ExternalInput")
    blobw_d = nc.dram_tensor("blobw", [P, BLOBW_F], F32, kind="ExternalInput")
    blob16a_d = nc.dram_tensor("blob16a", [CR, BLOB16A_F], BF16,
                               kind="ExternalInput")
    blob16c_d = nc.dram_tensor("blob16c", [CR, BLOB16C_F], BF16,
                               kind="ExternalInput")
    blob16b_d = nc.dram_tensor("blob16b", [2, BLOB16B_F], BF16,
                               kind="ExternalInput")
    blob16z_d = nc.dram_tensor("blob16z", [CR, BLOB16Z_F], BF16,
                               kind="ExternalInput")
    out_d = nc.dram_tensor("out", [ML, HID], F32, kind="ExternalOutput")

    with ExitStack() as ctx:
        tc = ctx.enter_context(tile.TileContext(nc))
        consts = ctx.enter_context(tc.tile_pool(name="consts", bufs=1))
        srow_p = ctx.enter_context(tc.tile_pool(name="srow", bufs=4))
        ps_d2 = ctx.enter_context(tc.tile_pool(name="ps_d2", bufs=2, space="PSUM"))
        ps_sm = ctx.enter_context(tc.tile_pool(name="ps_sm", bufs=4, space="PSUM"))
        small = ctx.enter_context(tc.tile_pool(name="small", bufs=16))
        gath = ctx.enter_context(tc.tile_pool(name="gath", bufs=4))
        outp = ctx.enter_context(tc.tile_pool(name="outp", bufs=4))

        # load order matters: the tiny bf16 rank blobs come first so PE can
        # start immediately; the big weight blob is only needed later.
        blob16z = consts.tile([CR, BLOB16Z_F], BF16, tag="blob16z")
        nc.sync.dma_start(blob16z[:], blob16z_d[:])
        blob16a = consts.tile([CR, BLOB16A_F], BF16, tag="blob16a")
        nc.sync.dma_start(blob16a[:], blob16a_d[:])
        blob16c = consts.tile([CR, BLOB16C_F], BF16, tag="blob16c")
        nc.sync.dma_start(blob16c[:], blob16c_d[:])
        blobh = consts.tile([P, BLOBH_F], F32, tag="blobh")
        nc.sync.dma_start(blobh[:], blobh_d[:])
        blob16b = consts.tile([2, BLOB16B_F], BF16, tag="blob16b")
        nc.sync.dma_start(blob16b[:], blob16b_d[:])
        blobw = consts.tile([P, BLOBW_F], F32, tag="blobw")
        nc.sync.dma_start(blobw[:], blobw_d[:])

        ident = blobh[0:P, OFF_ID:OFF_ID + P]
        b1c = lambda hc: blobh[0:P, OFF_B1C + hc:OFF_B1C + hc + 1]
        w1 = lambda c, hc: blobw[0:P, OFF_W1 + c * HID + hc * P:
                                 OFF_W1 + c * HID + (hc + 1) * P]
        w2 = lambda hc: blobw[0:P, OFF_W2 + hc * HID:OFF_W2 + (hc + 1) * HID]
        xst = lambda i: blobw[0:P, OFF_XST + i * P:OFF_XST + (i + 1) * P]
        def augp(j, i=1):
            if i == 0 and j == 0:
                return blob16z[0:CR, P:P + 512]
            if j < 2:
                return blob16a[0:CR, OFF_AP0 + j * 512:OFF_AP0 + (j + 1) * 512]
            return blob16c[0:CR, (j - 2) * 512:(j - 1) * 512]

        augq = lambda i: (blob16z[0:CR, 0:P] if i == 0 else
                          blob16a[0:CR, OFF_AQ + i * P:OFF_AQ + (i + 1) * P])
        ones2 = blob16b[0:2, OFF_ON2:OFF_ON2 + P]
        b2hl = blob16b[0:2, OFF_B2:OFF_B2 + HID]

        # per-tile tiles: Tile tracks dependencies at tile granularity, so
        # chunk-wide tensors would stall each tile's MLP on its neighbors
        catT = [[consts.tile([P, P], F32, tag=f"catT_{c}_{i}",
                             name=f"catT_{c}_{i}")
                 for i in range(T)] for c in range(2)]
        h1T = [[consts.tile([P, P], F32, tag=f"h1T_{hc}_{i}",
                            name=f"h1T_{hc}_{i}")
                for i in range(T)] for hc in range(2)]

        # ---- main loop over m-tiles, MLP interleaved per 4-tile chunk ----
        for i in range(T):
            # S = 2 q.p - |p|^2 for this tile's 128 queries x all 4096 points
            srow = srow_p.tile([P, N], F32, tag="srow")
            for j in range(N // 1024):
                ps = ps_d2.tile([P, 1024], F32, tag="ps")
                for h in range(2):
                    nc.tensor.matmul(ps[:, h * 512:(h + 1) * 512],
                                     lhsT=augq(i), rhs=augp(2 * j + h, i),
                                     start=True, stop=True)
                if i == 0:
                    # fine-grained drains on the first tile shorten the
                    # serial startup chain to the first MAX8
                    for h in range(2):
                        sl = slice((2 * j + h) * 512, (2 * j + h + 1) * 512)
                        nc.scalar.copy(srow[:, sl], ps[:, h * 512:(h + 1) * 512])
                else:
                    nc.scalar.copy(srow[:, j * 1024:(j + 1) * 1024], ps[:])

            # top-8 (we use top-3) by S descending == nearest by d2
            vals8 = small.tile([P, 8], F32, tag="vals8")
            idx8 = small.tile([P, 8], U32, tag="idx8")
            nc.vector.max(out=vals8[:], in_=srow[:])
            nc.vector.max_index(out=idx8[:], in_max=vals8[:], in_values=srow[:])

            # the matmul computes -d2 directly, so d2 = -vals
            d2s = small.tile([P, 3], F32, tag="d2s")
            nc.scalar.activation(d2s[:], vals8[:, 0:3],
                                 mybir.ActivationFunctionType.Identity,
                                 scale=-1.0)
            nc.vector.tensor_scalar_max(d2s[:], d2s[:], EPS)
            w3 = small.tile([P, 3], F32, tag="w3")
            nc.vector.reciprocal(w3[:], d2s[:])
            wsum = small.tile([P, 1], F32, tag="wsum")
            nc.vector.reduce_sum(wsum[:], w3[:], axis=mybir.AxisListType.X)
            winv = small.tile([P, 1], F32, tag="winv")
            nc.vector.reciprocal(winv[:], wsum[:])
            wn = small.tile([P, 3], F32, tag="wn")
            nc.scalar.activation(wn[:], w3[:],
                                 mybir.ActivationFunctionType.Copy,
                                 scale=winv[:])

            # gather the 3 neighbor rows; xg pools are deep enough that slots
            # are never reused, so each gather's only wait is the genuine DVE
            # idx8 dependency (keeps the SWDGE queue drain-free). Weighting
            # on ACT (per-partition scale).
            xgws = []
            for k in range(3):
                xg = gath.tile([P, C_IN], F32, tag=f"xg{k}", bufs=16,
                               name=f"xg{k}")
                gi = nc.gpsimd.indirect_dma_start(
                    out=xg[:], out_offset=None, in_=x_d[:],
                    in_offset=IndirectOffsetOnAxis(ap=idx8[:, k:k + 1], axis=0))
                # spread the gathers over the 4 SWDGE queues so the three
                # per-tile gathers run concurrently
                gi.ins.queue = f"qPoolDynamic{k or ''}"
                xgw = gath.tile([P, C_IN], F32, tag=f"xgw{k}", name=f"xgw{k}")
                nc.scalar.activation(xgw[:], xg[:],
                                     mybir.ActivationFunctionType.Copy,
                                     scale=wn[:, k:k + 1])
                xgws.append(xgw)

            # yT = sum_k transpose(xgw_k): pure accumulating PE transposes
            for h in range(2):
                pt = ps_sm.tile([P, P], F32, tag="sm")
                for k in range(3):
                    nc.tensor.matmul(pt[:], lhsT=xgws[k][:, h * P:(h + 1) * P],
                                     rhs=ident, is_transpose=True,
                                     start=(k == 0), stop=(k == 2))
                nc.scalar.copy(catT[h][i][:], pt[:])

            # MLP for this tile's 128 queries, emitted inline so PE/ACT load
            # stays even across tiles and nothing piles up at the end.
            for hc in range(2):
                ph = ps_sm.tile([P, 512], F32, tag="sm")
                for c in range(3):
                    rhs = catT[c][i][:] if c < 2 else xst(i)
                    nc.tensor.matmul(ph[:, :P], lhsT=w1(c, hc), rhs=rhs,
                                     start=(c == 0), stop=(c == 2))
                nc.scalar.activation(
                    h1T[hc][i][:], ph[:, :P],
                    mybir.ActivationFunctionType.Relu, bias=b1c(hc))
            po = ps_sm.tile([P, HID], F32, tag="sm")
            for hc in range(2):
                nc.tensor.matmul(
                    po[:], lhsT=h1T[hc][i][:],
                    rhs=w2(hc), start=(hc == 0), stop=False)
            nc.tensor.matmul(po[:], lhsT=ones2, rhs=b2hl,
                             start=False, stop=True)
            ob = outp.tile([P, HID], F32, tag="ob")
            nc.scalar.copy(ob[:], po[:])
            nc.sync.dma_start(out_d[i * P:(i + 1) * P, :], ob[:])

    if legalize:
        _legalize_waits(nc)
    return nc


def _split3(a):
    """fp32 -> (hi, mid, lo) bf16 triplet with hi+mid+lo ~= a to ~2^-25."""
    import ml_dtypes
    bf = ml_dtypes.bfloat16
    h = a.astype(bf)
    r = a - h.astype(np.float32)
    m = r.astype(bf)
    l = (r - m.astype(np.float32)).astype(bf)
    return h, m, l


def make_in_maps(x, pos, x_skip, pos_skip):
    """Host-side prep: split operands + per-core packed blobs."""
    import ml_dtypes
    bf = ml_dtypes.bfloat16

    x = np.ascontiguousarray(np.asarray(x, np.float32))
    pos = np.asarray(pos, np.float32)
    x_skip = np.asarray(x_skip, np.float32)
    pos_skip = np.asarray(pos_skip, np.float32)

    # The rank matmul computes S = -d2 = sum_c -(q_c - p_c)^2 with a 15-term
    # expansion per coordinate: hi/mid/lo bf16 splits of 2q_c, -q_c^2, p_c,
    # -p_c^2, ordered so the running partial cancels within each magnitude
    # level (keeps the fp32 PSUM accumulation noise near 1 ulp of d2's
    # scale instead of 1 ulp of |2q.p|).
    pc = [pos[:, c] for c in range(3)]
    psp = [_split3(p) for p in pc]                    # p_c splits
    npp = [_split3(-(p * p)) for p in pc]             # -p_c^2 splits
    onep = np.ones(N, np.float32).astype(bf)

    def coord_rows(qs2, nqq, ps, nps, onq, onp):
        """15 (q_row, p_row) pairs for one coordinate, small-partial order.
        qs2 = splits of 2q_c, nqq = splits of -q_c^2, ps = splits of p_c,
        nps = splits of -p_c^2, onq/onp = ones rows."""
        return [
            (qs2[0], ps[0]), (onq, nps[0]), (nqq[0], onp),      # hi level
            (qs2[0], ps[1]), (qs2[1], ps[0]), (onq, nps[1]),    # mid cross
            (nqq[1], onp), (qs2[1], ps[1]),
            (qs2[0], ps[2]), (qs2[2], ps[0]), (onq, nps[2]),    # lo cross
            (nqq[2], onp), (qs2[1], ps[2]), (qs2[2], ps[1]),
            (qs2[2], ps[2]),
        ]

    in_maps = []
    for core in range(N_CORES):
        sl = slice(core * ML, (core + 1) * ML)
        q = pos_skip[sl]
        qc = [q[:, c] for c in range(3)]
        qsp2 = [_split3(2.0 * qv) for qv in qc]
        nqq = [_split3(-(qv * qv)) for qv in qc]
        oneq = np.ones(ML, np.float32).astype(bf)

        aq_rows, ap_rows = [], []
        for c in range(3):
            for qr, pr in coord_rows(qsp2[c], nqq[c], psp[c], npp[c],
                                     oneq, onep):
                aq_rows.append(qr)
                ap_rows.append(pr)

        blobh = np.zeros((P, BLOBH_F), np.float32)
        blobh[0:P, OFF_ID:OFF_ID + P] = np.eye(P, dtype=np.float32)

        blobw = np.zeros((P, BLOBW_F), np.float32)
        blobw[0:P, OFF_XST:OFF_XST + ML] = x_skip[sl].T

        ap_full = np.stack(ap_rows)
        blob16a = np.zeros((CR, BLOB16A_F), bf)
        blob16a[0:CR, OFF_AQ:OFF_AQ + ML] = np.stack(aq_rows)
        blob16a[0:CR, OFF_AP0:OFF_AP0 + 1024] = ap_full[:, 0:1024]
        blob16c = np.ascontiguousarray(ap_full[:, 1024:N])

        blob16b = np.zeros((2, BLOB16B_F), bf)
        blob16b[0:2, OFF_ON2:OFF_ON2 + P] = np.ones((2, P), bf)

        blob16z = np.zeros((CR, BLOB16Z_F), bf)
        blob16z[0:CR, 0:P] = blob16a[0:CR, OFF_AQ:OFF_AQ + P]
        blob16z[0:CR, P:P + 512] = ap_full[:, 0:512]

        in_maps.append({"x": x, "blobh": blobh, "blobw": blobw,
                        "blob16a": blob16a, "blob16c": blob16c,
                        "blob16b": blob16b, "blob16z": blob16z})
    return in_maps


def fill_weights(in_maps, W1, b1, W2, b2):
    import ml_dtypes
    bf = ml_dtypes.bfloat16
    W1 = np.asarray(W1, np.float32)
    W2 = np.asarray(W2, np.float32)
    b1 = np.asarray(b1, np.float32).reshape(-1)
    b2 = np.asarray(b2, np.float32).reshape(-1)
    b2h = b2.astype(bf)
    b2l = (b2 - b2h.astype(np.float32)).astype(bf)
    for m in in_maps:
        blobw = m["blobw"]
        for c in range(3):
            blobw[0:P, OFF_W1 + c * HID:OFF_W1 + (c + 1) * HID] = \
                W1[c * P:(c + 1) * P, :]
        for hc in range(2):
            blobw[0:P, OFF_W2 + hc * HID:OFF_W2 + (hc + 1) * HID] = \
                W2[hc * P:(hc + 1) * P, :]
            m["blobh"][0:P, OFF_B1C + hc] = b1[hc * P:(hc + 1) * P]
        m["blob16b"][0:1, OFF_B2:OFF_B2 + HID] = b2h
        m["blob16b"][1:2, OFF_B2:OFF_B2 + HID] = b2l
    return in_maps


_NC_CACHE = {}


def kernel(x, pos, x_skip, pos_skip, W1, b1, W2, b2):
    from concourse.bass_utils import run_bass_kernel_spmd

    if "nc" not in _NC_CACHE:
        _NC_CACHE["nc"] = build_program()
    nc = _NC_CACHE["nc"]

    in_maps = make_in_maps(x, pos, x_skip, pos_skip)
    fill_weights(in_maps, W1, b1, W2, b2)

    res = run_bass_kernel_spmd(nc, in_maps, list(range(N_CORES))).results
    out = np.concatenate([res[c]["out"] for c in range(N_CORES)], axis=0)
    return out.astype(np.float32)
